# revision 82
# baseline (speedup 1.0000x reference)
"""Trainium2 Bass kernel for nn_AttnClassifier (dense transformer, N=8192 patches).

Sharding: N (patch) dim row-blocked across 8 NeuronCores. Each core:
  fc1+fc2 for its 1024 patches -> h2 slice (both [n,c] and [c,n] layouts),
  local Q rows + local K columns, AllGather {h2, k} across cores,
  then flash-style row-block attention: eT = k^T q (transposed energy,
  [n,m] layout), P = exp(eT), AV = h2^T P accumulated over all n, rowsum
  via DVE accumulation + ones-matmul partition reduce.  Per-core output is
  a pair of per-channel partial sums; the host combines them into the
  final 2-logit head (a 256->2 matvec).

Matmuls run in float32r (full PE speed at free dim >= 256, ~1.6e-4 rel err).
"""

import numpy as np

import concourse.bass as bass
import concourse.tile as tile
from concourse.tile import add_dep_helper
from concourse import bacc, mybir
from concourse import bass_utils

P = 128
D, H, C = 1024, 512, 256
N = 8192
NCORES = 8
M = N // NCORES           # 1024 query rows per core
NT = N // P               # 64 n-tiles (key/value tiles) total
NT_LOC = M // P           # 8 n-tiles per rank
F32 = mybir.dt.float32
F32R = mybir.dt.float32r
BF16 = mybir.dt.bfloat16
AF = mybir.ActivationFunctionType
ALU = mybir.AluOpType

HSHARD = M * C            # floats in the h2 ([n, c]) shard
KSHARD = C * M            # floats in the k ([c, n_local]) shard
SHARD = HSHARD + KSHARD


def _body(nc, tc, ins, p1_ap, p2_ap, collective):
    with (
        tc.tile_pool(name="const", bufs=1) as cpool,
        tc.tile_pool(name="persist", bufs=1) as pers,
        tc.tile_pool(name="dram", bufs=1, space="DRAM") as dram,
    ):
        # ---- persistent per-core tensors
        h2T = [pers.tile([P, M], F32R, name=f"h2T{i}") for i in range(C // P)]
        qT = [pers.tile([P, M], BF16, name=f"qT{i}") for i in range(C // P)]
        S = pers.tile([P, M], F32, name="S")          # per-subrow partial rowsums

        # ---- DRAM bounce buffers for the two AllGathers (k first, then h2)
        aginK = dram.tile([KSHARD], BF16, name="aginK")
        aginH = dram.tile([HSHARD], BF16, name="aginH")
        agoutK = dram.tile(
            [NCORES * KSHARD], BF16, name="agoutK",
            addr_space="Shared" if collective else "Local",
        )
        agoutH = dram.tile(
            [NCORES * HSHARD], BF16, name="agoutH",
            addr_space="Shared" if collective else "Local",
        )

        with (
            tc.tile_pool(name="attn", bufs=2) as apool,
            tc.tile_pool(name="kr", bufs=3) as krpool,
            tc.tile_pool(name="epsum", bufs=4, space="PSUM") as eps,
            tc.tile_pool(name="avpsum", bufs=1, space="PSUM") as avp,
        ):
          kr_tiles = {}
          kr_dmas = []

          def load_kr(r, split_head=False):
              kr = [
                  krpool.tile([P, M], BF16, name=f"kr{ot}")
                  for ot in range(C // P)
              ]
              for ot in range(C // P):
                  base = (r * C // P + ot) * P * M
                  src = agoutK[base : base + P * M].rearrange("(p f) -> p f", p=P)
                  if split_head:
                      hd = 256
                      d = nc.sync.dma_start(kr[ot][:, :hd], src[:, :hd])
                      kr_dmas.append(d)
                      d = nc.sync.dma_start(kr[ot][:, hd:], src[:, hd:])
                      kr_dmas.append(d)
                  else:
                      d = nc.sync.dma_start(kr[ot][:], src)
                      kr_dmas.append(d)
              kr_tiles[r] = kr

          with tc.tile_pool(name="fcbuf", bufs=1) as fcp:
              # ===== phase 1: fc1 (h1T[h, m] = relu(W1T^T @ xT + b1)), bf16 inputs.
              # d-tile outer so matmuls start as soon as the first DMAs land;
              # all 8 PSUM banks hold the 4x2 output accumulation.
              xt = [fcp.tile([P, M], BF16, name=f"xt{i}") for i in range(D // P)]
              w1t = [fcp.tile([P, H], BF16, name=f"w1t{i}") for i in range(D // P)]
              for i in range(D // P):
                  nc.sync.dma_start(w1t[i][:], ins["w1t"][bass.ts(i, P), :])
                  nc.gpsimd.dma_start(xt[i][:], ins["xT"][bass.ts(i, P), :])

              # ---- constants / weights staged in SBUF (queued behind tile 0)
              w2t = [cpool.tile([P, C], F32R, name=f"w2t{i}") for i in range(H // P)]
              for i in range(H // P):
                  nc.sync.dma_start(w2t[i][:], ins["w2t"][bass.ts(i, P), :])
              wqt = [cpool.tile([P, C], F32R, name=f"wqt{i}") for i in range(C // P)]
              wkt = [cpool.tile([P, C], F32R, name=f"wkt{i}") for i in range(C // P)]
              for i in range(C // P):
                  nc.sync.dma_start(wqt[i][:], ins["wqt"][bass.ts(i, P), :])
                  nc.sync.dma_start(wkt[i][:], ins["wkt"][bass.ts(i, P), :])
              b1sb = cpool.tile([P, H // P], F32, name="b1sb")
              nc.sync.dma_start(b1sb[:], ins["b1f"].rearrange("(t p) -> p t", p=P))
              b2sb = cpool.tile([P, C // P], F32, name="b2sb")
              nc.sync.dma_start(b2sb[:], ins["b2f"].rearrange("(t p) -> p t", p=P))
              bqsb = cpool.tile([P, C // P], F32, name="bqsb")
              nc.sync.dma_start(bqsb[:], ins["bqf"].rearrange("(t p) -> p t", p=P))
              bksb = cpool.tile([P, C // P], F32, name="bksb")
              nc.sync.dma_start(bksb[:], ins["bkf"].rearrange("(t p) -> p t", p=P))
              b2r = cpool.tile([1, C], F32R, name="b2r")
              nc.sync.dma_start(b2r[:], ins["b2r"][:])
              ones_r = cpool.tile([1, P], F32R, name="ones_r")
              nc.sync.dma_start(ones_r[:], ins["ones_r"][:])
              ones_f = cpool.tile([P, 1], F32, name="ones_f")
              nc.sync.dma_start(ones_f[:], ins["ones_f"][:])

              h1t = [fcp.tile([P, M], F32R, name=f"h1t{i}") for i in range(H // P)]
              if True:
                  _pe = [
                      eps.tile([P, 512], F32, name=f"ps1_{j}", tag="e_ps")
                      for j in range(4)
                  ]
                  _pa = avp.tile([P, M], F32, name="av0_fc1", tag="av0")
                  _pb = avp.tile([P, M], F32, name="av1_fc1", tag="av1")
                  ps1 = _pe + [
                      _pa[:, :512], _pa[:, 512:], _pb[:, :512], _pb[:, 512:]
                  ]
                  for dt_ in range(D // P - 1):
                      for ht in range(H // P):
                          for mh in range(M // 512):
                              nc.tensor.matmul(
                                  ps1[ht * 2 + mh][:],
                                  w1t[dt_][:, bass.ts(ht, P)],
                                  xt[dt_][:, bass.ts(mh, 512)],
                                  start=(dt_ == 0),
                                  stop=False,
                              )
                  dt_ = D // P - 1
                  for mh in range(M // 512):
                      for ht in range(H // P):
                          nc.tensor.matmul(
                              ps1[ht * 2 + mh][:],
                              w1t[dt_][:, bass.ts(ht, P)],
                              xt[dt_][:, bass.ts(mh, 512)],
                              start=False,
                              stop=True,
                          )
                          eng = nc.vector if ht % 2 == 0 else nc.scalar
                          if ht % 2 == 0:
                              nc.vector.tensor_scalar(
                                  h1t[ht][:, bass.ts(mh, 512)], ps1[ht * 2 + mh][:],
                                  b1sb[:, ht : ht + 1], 0.0, ALU.add, ALU.max,
                              )
                          else:
                              nc.scalar.activation(
                                  h1t[ht][:, bass.ts(mh, 512)], ps1[ht * 2 + mh][:],
                                  AF.Relu, bias=b1sb[:, ht : ht + 1],
                              )

              if True:
                  # ===== phase 2a: h2T[c, m] = relu(W2T^T @ h1T + b2)  (ACT)
                  for ct in range(C // P):
                      for mh in range(M // 512):
                          ps = eps.tile([P, 512], F32, name="ps_fc2b", tag="e_ps")
                          for ht in range(H // P):
                              nc.tensor.matmul(
                                  ps[:],
                                  w2t[ht][:, bass.ts(ct, P)],
                                  h1t[ht][:, bass.ts(mh, 512)],
                                  start=(ht == 0),
                                  stop=(ht == H // P - 1),
                              )
                          nc.scalar.activation(
                              h2T[ct][:, bass.ts(mh, 512)], ps[:], AF.Relu,
                              bias=b2sb[:, ct : ct + 1],
                          )

                  # ===== phase 2b: k_own[o, m] = WkT^T @ h2T + bk -> aginK, AG(k)
                  for ot in range(C // P):
                      ko_ps = [
                          eps.tile([P, 512], F32, name=f"ps_k{i}", tag="e_ps")
                          for i in range(M // 512)
                      ]
                      for mh in range(M // 512):
                          for ct in range(C // P):
                              nc.tensor.matmul(
                                  ko_ps[mh][:],
                                  wkt[ct][:, bass.ts(ot, P)],
                                  h2T[ct][:, bass.ts(mh, 512)],
                                  start=(ct == 0),
                                  stop=(ct == C // P - 1),
                              )
                      k_own = fcp.tile([P, M], BF16, name="k_own", bufs=2)
                      for mh in range(M // 512):
                          nc.vector.tensor_scalar_add(
                              k_own[:, bass.ts(mh, 512)], ko_ps[mh][:],
                              bksb[:, ot : ot + 1],
                          )
                      nc.sync.dma_start(
                          aginK[ot * P * M : (ot + 1) * P * M].rearrange(
                              "(p f) -> p f", p=P
                          ),
                          k_own[:],
                      )
                  if collective:
                      nc.gpsimd.collective_compute(
                          "AllGather", ALU.bypass,
                          replica_groups=[list(range(NCORES))],
                          ins=[aginK.opt()], outs=[agoutK.opt()],
                      )
                  else:
                      nc.sync.dma_start(
                          agoutK[:KSHARD].rearrange("(p f) -> p f", p=P),
                          aginK[:].rearrange("(p f) -> p f", p=P),
                      )

                  load_kr(0)
                  load_kr(1)

                  # ===== phase 4: h2_own[n, c] = relu(h1T^T @ W2T + b2) -> aginH, AG(h2)
                  h2all = fcp.tile([P, NT_LOC * C], BF16, name="h2all")
                  for mt in range(NT_LOC):
                      ps_full = eps.tile([P, 512], F32, name="ps_fc2a", tag="e_ps")
                      ps = ps_full[:, :C]
                      for ht in range(H // P):
                          nc.tensor.matmul(
                              ps[:],
                              h1t[ht][:, bass.ts(mt, P)],
                              w2t[ht][:],
                              start=(ht == 0),
                              stop=False,
                          )
                      nc.tensor.matmul(ps[:], ones_r[:], b2r[:], start=False, stop=True)
                      nc.vector.tensor_scalar_max(
                          h2all[:, bass.ts(mt, C)], ps[:], 0.0
                      )
                  nc.sync.dma_start(
                      aginH[:].rearrange("(t p f) -> p t f", t=NT_LOC, p=P),
                      h2all[:].rearrange("p (t f) -> p t f", t=NT_LOC),
                  )
                  if collective:
                      nc.gpsimd.collective_compute(
                          "AllGather", ALU.bypass,
                          replica_groups=[list(range(NCORES))],
                          ins=[aginH.opt()], outs=[agoutH.opt()],
                      )
                  else:
                      nc.sync.dma_start(
                          agoutH[:HSHARD].rearrange("(p f) -> p f", p=P),
                          aginH[:].rearrange("(p f) -> p f", p=P),
                      )

                  # ===== phase 3: qT[o, m] = WqT^T @ h2T + bq  (DVE epilogue)
                  for ot in range(C // P):
                      for mh in range(M // 512):
                          ps = eps.tile([P, 512], F32, name="ps_q", tag="e_ps")
                          for ct in range(C // P):
                              nc.tensor.matmul(
                                  ps[:],
                                  wqt[ct][:, bass.ts(ot, P)],
                                  h2T[ct][:, bass.ts(mh, 512)],
                                  start=(ct == 0),
                                  stop=(ct == C // P - 1),
                              )
                          nc.scalar.activation(
                              qT[ot][:, bass.ts(mh, 512)], ps[:], AF.Identity,
                              bias=bqsb[:, ot : ot + 1],
                          )

                  # ===== PE warm-up filler: keep the tensor engine busy through
                  # the AllGather wait so attention starts at full clock rate.
                  scratch_ps = eps.tile([P, 512], F32, name="warm_ps", tag="e_ps", bufs=4)
                  for w_ in range(24):
                      nc.tensor.matmul(
                          scratch_ps[:],
                          wqt[w_ % 2][:, 0:P],
                          h2T[w_ % 2][:, 0:512],
                          start=(w_ == 0),
                          stop=(w_ == 23),
                      )
                  warm_sb = apool.tile([1, 4], F32, name="warm_sb", bufs=1)
                  nc.vector.tensor_copy(out=warm_sb[:], in_=scratch_ps[0:1, 0:4])
                  warm_dram = dram.tile([4], F32, name="warm_dram")
                  nc.sync.dma_start(warm_dram[:].rearrange("(p f) -> p f", p=1), warm_sb[:])

          # ================= phase 5: attention over all 64 key tiles,
          # AV matmuls software-pipelined one tile behind eT/exp; pt bufs=8
          # lets eT/exp run ahead while the h2 AllGather finishes.
          nc.vector.memset(S[:], 0.0)
          if True:
              p2sb = apool.tile([P, C // P], F32, name="p2sb", bufs=1)
              for cc in range(C // P):
                  nc.vector.reduce_sum(
                      p2sb[:, cc : cc + 1], h2T[cc][:].bitcast(F32),
                      axis=mybir.AxisListType.X,
                  )
              nc.sync.dma_start(p2_ap.rearrange("(t p) -> p t", p=P), p2sb[:])
              av_ps = [avp.tile([P, M], F32, name=f"av{cc}", tag=f"av{cc}") for cc in range(C // P)]

              def emit_av(pt_, h2r_, t_, tg_):
                  for cc in range(C // P):
                      for mh in range(M // 512):
                          nc.tensor.matmul(
                              av_ps[cc][:, bass.ts(mh, 512)],
                              h2r_[:, t_ * C + cc * P : t_ * C + (cc + 1) * P],
                              pt_[:, bass.ts(mh, 512)],
                              start=(tg_ == 0),
                              stop=(tg_ == NT - 1),
                          )

              AV_DELAY = 10
              pending = []
              for r in range(NCORES):
                  if r not in kr_tiles:
                      load_kr(r)
                  kr = kr_tiles[r]
                  h2r = krpool.tile([P, NT_LOC * C], BF16, name="h2r")
                  h2r_dma = nc.sync.dma_start(
                      h2r[:].rearrange("p (t f) -> p t f", t=NT_LOC),
                      agoutH[r * HSHARD : (r + 1) * HSHARD].rearrange(
                          "(t p f) -> p t f", t=NT_LOC, p=P
                      ),
                  )
                  if r == 0:
                      # keep the first eT's k loads ahead of the 1MB h2 load
                      for d_ in kr_dmas[:4]:
                          add_dep_helper(h2r_dma.ins, d_.ins, sync=False,
                                         reason="rank0 h2r after entry kr loads")
                  for t in range(NT_LOC):
                      tg = r * NT_LOC + t
                      pt = apool.tile([P, M], BF16, name="pt", bufs=16)
                      for mh in range(M // 512):
                          e_ps = eps.tile([P, 512], F32, name="e_ps", tag="e_ps", bufs=4)
                          for ot in range(C // P):
                              nc.tensor.matmul(
                                  e_ps[:],
                                  kr[ot][:, bass.ts(t, P)],
                                  qT[ot][:, bass.ts(mh, 512)],
                                  start=(ot == 0),
                                  stop=(ot == C // P - 1),
                              )
                          nc.scalar.activation(pt[:, bass.ts(mh, 512)], e_ps[:], AF.Exp)
                      nc.vector.tensor_tensor(S[:], S[:], pt[:], ALU.add)
                      pending.append((pt, h2r, t, tg))
                      if len(pending) > AV_DELAY:
                          emit_av(*pending.pop(0))
              for args in pending:
                  emit_av(*args)

              # ===== epilogue: rowsum -> reciprocal -> scale -> partial sums
              rs_full = eps.tile([P, 512], F32, name="rs_ps", tag="e_ps", bufs=4)
              rs_full2 = eps.tile([P, 512], F32, name="rs_ps2", tag="e_ps", bufs=4)
              rs_halves = [rs_full[0:1, :], rs_full2[0:1, :]]
              for mh in range(M // 512):
                  nc.tensor.matmul(
                      rs_halves[mh][:],
                      ones_f[:],
                      S[:, bass.ts(mh, 512)],
                      start=True,
                      stop=True,
                  )
              recip = apool.tile([1, M], F32, name="recip", bufs=1)
              recipb = apool.tile([P, M], F32, name="recipb", bufs=1)
              for mh in range(M // 512):
                  nc.vector.reciprocal(recip[:, bass.ts(mh, 512)], rs_halves[mh][:])
                  nc.gpsimd.partition_broadcast(
                      recipb[:, bass.ts(mh, 512)], recip[:, bass.ts(mh, 512)]
                  )

              p1sb = apool.tile([P, C // P], F32, name="p1sb", bufs=1)
              p1h = apool.tile([P, 2 * (C // P)], F32, name="p1h", bufs=1)
              for cc in range(C // P):
                  avn = apool.tile([P, M], F32, name="avn", bufs=2)
                  for mh in range(M // 512):
                      j = cc * 2 + mh
                      nc.vector.tensor_tensor(
                          avn[:, bass.ts(mh, 512)],
                          av_ps[cc][:, bass.ts(mh, 512)],
                          recipb[:, bass.ts(mh, 512)], ALU.mult,
                      )
                      nc.vector.reduce_sum(
                          p1h[:, j : j + 1], avn[:, bass.ts(mh, 512)],
                          axis=mybir.AxisListType.X,
                      )
                  nc.vector.tensor_tensor(
                      p1sb[:, cc : cc + 1], p1h[:, cc * 2 : cc * 2 + 1],
                      p1h[:, cc * 2 + 1 : cc * 2 + 2], ALU.add,
                  )
              nc.sync.dma_start(p1_ap.rearrange("(t p) -> p t", p=P), p1sb[:])


def build_nc(collective=True, repeat=1):
    nc = bacc.Bacc("TRN2", target_bir_lowering=False, debug=False, num_devices=NCORES)
    ins = {}

    def di(name, shape, dt):
        ins[name] = nc.dram_tensor(name, list(shape), dt, kind="ExternalInput").ap()

    di("xT", (D, M), BF16)
    di("w1t", (D, H), BF16)
    di("w2t", (H, C), F32R)
    di("wqt", (C, C), F32R)
    di("wkt", (C, C), F32R)
    di("b2r", (1, C), F32R)
    di("ones_r", (1, P), F32R)
    di("ones_f", (P, 1), F32)
    di("b1f", (H,), F32)
    di("b2f", (C,), F32)
    di("bqf", (C,), F32)
    di("bkf", (C,), F32)
    p1_ap = nc.dram_tensor("partial_av", [C], F32, kind="ExternalOutput").ap()
    p2_ap = nc.dram_tensor("partial_res", [C], F32, kind="ExternalOutput").ap()

    with tile.TileContext(nc) as tc:
        for _ in range(repeat):
            _body(nc, tc, ins, p1_ap, p2_ap, collective)
    nc.compile()
    return nc


_CACHE = {}


def _get_nc(collective=True, repeat=1):
    key = (collective, repeat)
    if key not in _CACHE:
        _CACHE[key] = build_nc(collective=collective, repeat=repeat)
    return _CACHE[key]


def make_in_maps(x, W1, b1, W2, b2, Wq, bq, Wk, bk):
    import ml_dtypes
    xT = np.ascontiguousarray(np.asarray(x, np.float32)[0].T.astype(ml_dtypes.bfloat16))
    w1t = np.ascontiguousarray(np.asarray(W1, np.float32).T.astype(ml_dtypes.bfloat16))
    w2t = np.ascontiguousarray(np.asarray(W2, np.float32).T)        # (H, C)
    wqt = np.ascontiguousarray(np.asarray(Wq, np.float32).T)        # (C, C)
    wkt = np.ascontiguousarray(np.asarray(Wk, np.float32).T)        # (C, C)
    common = {
        "w1t": w1t, "w2t": w2t, "wqt": wqt, "wkt": wkt,
        "b2r": np.asarray(b2, np.float32).reshape(1, C),
        "ones_r": np.ones((1, P), np.float32),
        "ones_f": np.ones((P, 1), np.float32),
        "b1f": np.asarray(b1, np.float32),
        "b2f": np.asarray(b2, np.float32),
        "bqf": np.asarray(bq, np.float32),
        "bkf": np.asarray(bk, np.float32),
    }
    return [
        {"xT": np.ascontiguousarray(xT[:, r * M : (r + 1) * M]), **common}
        for r in range(NCORES)
    ]


def finish(results, gamma, W3, b3):
    p1 = np.sum([r["partial_av"] for r in results], axis=0, dtype=np.float64)
    p2 = np.sum([r["partial_res"] for r in results], axis=0, dtype=np.float64)
    g = float(np.asarray(gamma).reshape(-1)[0])
    x2 = ((g * p1 + p2) / N).astype(np.float32)
    logits = x2 @ np.asarray(W3, np.float32).T + np.asarray(b3, np.float32)
    return logits[None, :].astype(np.float32)


def kernel(x, W1, b1, W2, b2, Wq, bq, Wk, bk, gamma, W3, b3):
    nc = _get_nc(collective=True, repeat=1)
    in_maps = make_in_maps(x, W1, b1, W2, b2, Wq, bq, Wk, bk)
    res = bass_utils.run_bass_kernel_spmd(
        nc, in_maps, core_ids=list(range(NCORES)), trace=False
    )
    return finish(res.results, gamma, W3, b3)



# revision 89
# speedup vs baseline: 1.0020x; 1.0020x over previous
"""Trainium2 Bass kernel for nn_AttnClassifier (dense transformer, N=8192 patches).

Sharding: N (patch) dim row-blocked across 8 NeuronCores. Each core:
  fc1+fc2 for its 1024 patches -> h2 slice (both [n,c] and [c,n] layouts),
  local Q rows + local K columns, AllGather {h2, k} across cores,
  then flash-style row-block attention: eT = k^T q (transposed energy,
  [n,m] layout), P = exp(eT), AV = h2^T P accumulated over all n, rowsum
  via DVE accumulation + ones-matmul partition reduce.  Per-core output is
  a pair of per-channel partial sums; the host combines them into the
  final 2-logit head (a 256->2 matvec).

Matmuls run in float32r (full PE speed at free dim >= 256, ~1.6e-4 rel err).
"""

import numpy as np

import concourse.bass as bass
import concourse.tile as tile
from concourse.tile import add_dep_helper
from concourse import bacc, mybir
from concourse import bass_utils

P = 128
D, H, C = 1024, 512, 256
N = 8192
NCORES = 8
M = N // NCORES           # 1024 query rows per core
NT = N // P               # 64 n-tiles (key/value tiles) total
NT_LOC = M // P           # 8 n-tiles per rank
F32 = mybir.dt.float32
F32R = mybir.dt.float32r
BF16 = mybir.dt.bfloat16
AF = mybir.ActivationFunctionType
ALU = mybir.AluOpType

HSHARD = M * C            # floats in the h2 ([n, c]) shard
KSHARD = C * M            # floats in the k ([c, n_local]) shard
SHARD = HSHARD + KSHARD


def _body(nc, tc, ins, p1_ap, p2_ap, collective):
    with (
        tc.tile_pool(name="const", bufs=1) as cpool,
        tc.tile_pool(name="persist", bufs=1) as pers,
        tc.tile_pool(name="dram", bufs=1, space="DRAM") as dram,
    ):
        # ---- persistent per-core tensors
        h2T = [pers.tile([P, M], F32R, name=f"h2T{i}") for i in range(C // P)]
        qT = [pers.tile([P, M], BF16, name=f"qT{i}") for i in range(C // P)]
        S = pers.tile([P, M], F32, name="S")          # per-subrow partial rowsums

        # ---- DRAM bounce buffers for the two AllGathers (k first, then h2)
        aginK = dram.tile([KSHARD], BF16, name="aginK")
        aginH = dram.tile([HSHARD], BF16, name="aginH")
        agoutK = dram.tile(
            [NCORES * KSHARD], BF16, name="agoutK",
            addr_space="Shared" if collective else "Local",
        )
        agoutH = dram.tile(
            [NCORES * HSHARD], BF16, name="agoutH",
            addr_space="Shared" if collective else "Local",
        )

        with (
            tc.tile_pool(name="attn", bufs=2) as apool,
            tc.tile_pool(name="kr", bufs=3) as krpool,
            tc.tile_pool(name="epsum", bufs=4, space="PSUM") as eps,
            tc.tile_pool(name="avpsum", bufs=1, space="PSUM") as avp,
        ):
          kr_tiles = {}
          kr_dmas = []

          def load_kr(r, split_head=False):
              kr = [
                  krpool.tile([P, M], BF16, name=f"kr{ot}")
                  for ot in range(C // P)
              ]
              for ot in range(C // P):
                  base = (r * C // P + ot) * P * M
                  src = agoutK[base : base + P * M].rearrange("(p f) -> p f", p=P)
                  if split_head:
                      hd = 256
                      d = nc.sync.dma_start(kr[ot][:, :hd], src[:, :hd])
                      kr_dmas.append(d)
                      d = nc.sync.dma_start(kr[ot][:, hd:], src[:, hd:])
                      kr_dmas.append(d)
                  else:
                      d = nc.sync.dma_start(kr[ot][:], src)
                      kr_dmas.append(d)
              kr_tiles[r] = kr

          with tc.tile_pool(name="fcbuf", bufs=1) as fcp:
              # ===== phase 1: fc1 (h1T[h, m] = relu(W1T^T @ xT + b1)), bf16 inputs.
              # d-tile outer so matmuls start as soon as the first DMAs land;
              # all 8 PSUM banks hold the 4x2 output accumulation.
              xt = [fcp.tile([P, M], BF16, name=f"xt{i}") for i in range(D // P)]
              w1t = [fcp.tile([P, H], BF16, name=f"w1t{i}") for i in range(D // P)]
              for i in range(D // P):
                  nc.sync.dma_start(w1t[i][:], ins["w1t"][bass.ts(i, P), :])
                  nc.gpsimd.dma_start(xt[i][:], ins["xT"][bass.ts(i, P), :])

              # ---- constants / weights staged in SBUF (queued behind tile 0)
              w2t = [cpool.tile([P, C], F32R, name=f"w2t{i}") for i in range(H // P)]
              for i in range(H // P):
                  nc.sync.dma_start(w2t[i][:], ins["w2t"][bass.ts(i, P), :])
              wqt = [cpool.tile([P, C], F32R, name=f"wqt{i}") for i in range(C // P)]
              wkt = [cpool.tile([P, C], F32R, name=f"wkt{i}") for i in range(C // P)]
              for i in range(C // P):
                  nc.sync.dma_start(wqt[i][:], ins["wqt"][bass.ts(i, P), :])
                  nc.sync.dma_start(wkt[i][:], ins["wkt"][bass.ts(i, P), :])
              b1sb = cpool.tile([P, H // P], F32, name="b1sb")
              nc.sync.dma_start(b1sb[:], ins["b1f"].rearrange("(t p) -> p t", p=P))
              b2sb = cpool.tile([P, C // P], F32, name="b2sb")
              nc.sync.dma_start(b2sb[:], ins["b2f"].rearrange("(t p) -> p t", p=P))
              bqsb = cpool.tile([P, C // P], F32, name="bqsb")
              nc.sync.dma_start(bqsb[:], ins["bqf"].rearrange("(t p) -> p t", p=P))
              bksb = cpool.tile([P, C // P], F32, name="bksb")
              nc.sync.dma_start(bksb[:], ins["bkf"].rearrange("(t p) -> p t", p=P))
              b2r = cpool.tile([1, C], F32R, name="b2r")
              nc.sync.dma_start(b2r[:], ins["b2r"][:])
              ones_r = cpool.tile([1, P], F32R, name="ones_r")
              nc.sync.dma_start(ones_r[:], ins["ones_r"][:])
              ones_f = cpool.tile([P, 1], F32, name="ones_f")
              nc.sync.dma_start(ones_f[:], ins["ones_f"][:])

              h1t = [fcp.tile([P, M], F32R, name=f"h1t{i}") for i in range(H // P)]
              if True:
                  _pe = [
                      eps.tile([P, 512], F32, name=f"ps1_{j}", tag="e_ps")
                      for j in range(4)
                  ]
                  _pa = avp.tile([P, M], F32, name="av0_fc1", tag="av0")
                  _pb = avp.tile([P, M], F32, name="av1_fc1", tag="av1")
                  ps1 = _pe + [
                      _pa[:, :512], _pa[:, 512:], _pb[:, :512], _pb[:, 512:]
                  ]
                  for dt_ in range(D // P - 1):
                      for ht in range(H // P):
                          for mh in range(M // 512):
                              nc.tensor.matmul(
                                  ps1[ht * 2 + mh][:],
                                  w1t[dt_][:, bass.ts(ht, P)],
                                  xt[dt_][:, bass.ts(mh, 512)],
                                  start=(dt_ == 0),
                                  stop=False,
                              )
                  dt_ = D // P - 1
                  for mh in range(M // 512):
                      for ht in range(H // P):
                          nc.tensor.matmul(
                              ps1[ht * 2 + mh][:],
                              w1t[dt_][:, bass.ts(ht, P)],
                              xt[dt_][:, bass.ts(mh, 512)],
                              start=False,
                              stop=True,
                          )
                          eng = nc.vector if ht % 2 == 0 else nc.scalar
                          if ht % 2 == 0:
                              nc.vector.tensor_scalar(
                                  h1t[ht][:, bass.ts(mh, 512)], ps1[ht * 2 + mh][:],
                                  b1sb[:, ht : ht + 1], 0.0, ALU.add, ALU.max,
                              )
                          else:
                              nc.scalar.activation(
                                  h1t[ht][:, bass.ts(mh, 512)], ps1[ht * 2 + mh][:],
                                  AF.Relu, bias=b1sb[:, ht : ht + 1],
                              )

              if True:
                  # ===== phase 2a: h2T[c, m] = relu(W2T^T @ h1T + b2)  (ACT)
                  for ct in range(C // P):
                      for mh in range(M // 512):
                          ps = eps.tile([P, 512], F32, name="ps_fc2b", tag="e_ps")
                          for ht in range(H // P):
                              nc.tensor.matmul(
                                  ps[:],
                                  w2t[ht][:, bass.ts(ct, P)],
                                  h1t[ht][:, bass.ts(mh, 512)],
                                  start=(ht == 0),
                                  stop=(ht == H // P - 1),
                              )
                          nc.vector.tensor_scalar(
                              h2T[ct][:, bass.ts(mh, 512)], ps[:],
                              b2sb[:, ct : ct + 1], 0.0, ALU.add, ALU.max,
                          )

                  # ===== phase 2b: k_own[o, m] = WkT^T @ h2T + bk -> aginK, AG(k)
                  for ot in range(C // P):
                      ko_ps = [
                          eps.tile([P, 512], F32, name=f"ps_k{i}", tag="e_ps")
                          for i in range(M // 512)
                      ]
                      for mh in range(M // 512):
                          for ct in range(C // P):
                              nc.tensor.matmul(
                                  ko_ps[mh][:],
                                  wkt[ct][:, bass.ts(ot, P)],
                                  h2T[ct][:, bass.ts(mh, 512)],
                                  start=(ct == 0),
                                  stop=(ct == C // P - 1),
                              )
                      k_own = fcp.tile([P, M], BF16, name="k_own", bufs=2)
                      for mh in range(M // 512):
                          nc.scalar.activation(
                              k_own[:, bass.ts(mh, 512)], ko_ps[mh][:],
                              AF.Identity, bias=bksb[:, ot : ot + 1],
                          )
                      nc.sync.dma_start(
                          aginK[ot * P * M : (ot + 1) * P * M].rearrange(
                              "(p f) -> p f", p=P
                          ),
                          k_own[:],
                      )
                  if collective:
                      nc.gpsimd.collective_compute(
                          "AllGather", ALU.bypass,
                          replica_groups=[list(range(NCORES))],
                          ins=[aginK.opt()], outs=[agoutK.opt()],
                      )
                  else:
                      nc.sync.dma_start(
                          agoutK[:KSHARD].rearrange("(p f) -> p f", p=P),
                          aginK[:].rearrange("(p f) -> p f", p=P),
                      )

                  load_kr(0)
                  load_kr(1)

                  # ===== phase 4: h2_own[n, c] = relu(h1T^T @ W2T + b2) -> aginH, AG(h2)
                  h2all = fcp.tile([P, NT_LOC * C], BF16, name="h2all")
                  for mt in range(NT_LOC):
                      ps_full = eps.tile([P, 512], F32, name="ps_fc2a", tag="e_ps")
                      ps = ps_full[:, :C]
                      for ht in range(H // P):
                          nc.tensor.matmul(
                              ps[:],
                              h1t[ht][:, bass.ts(mt, P)],
                              w2t[ht][:],
                              start=(ht == 0),
                              stop=False,
                          )
                      nc.tensor.matmul(ps[:], ones_r[:], b2r[:], start=False, stop=True)
                      nc.vector.tensor_scalar_max(
                          h2all[:, bass.ts(mt, C)], ps[:], 0.0
                      )
                  nc.sync.dma_start(
                      aginH[:].rearrange("(t p f) -> p t f", t=NT_LOC, p=P),
                      h2all[:].rearrange("p (t f) -> p t f", t=NT_LOC),
                  )
                  if collective:
                      nc.gpsimd.collective_compute(
                          "AllGather", ALU.bypass,
                          replica_groups=[list(range(NCORES))],
                          ins=[aginH.opt()], outs=[agoutH.opt()],
                      )
                  else:
                      nc.sync.dma_start(
                          agoutH[:HSHARD].rearrange("(p f) -> p f", p=P),
                          aginH[:].rearrange("(p f) -> p f", p=P),
                      )

                  # ===== phase 3: qT[o, m] = WqT^T @ h2T + bq  (DVE epilogue)
                  for ot in range(C // P):
                      for mh in range(M // 512):
                          ps = eps.tile([P, 512], F32, name="ps_q", tag="e_ps")
                          for ct in range(C // P):
                              nc.tensor.matmul(
                                  ps[:],
                                  wqt[ct][:, bass.ts(ot, P)],
                                  h2T[ct][:, bass.ts(mh, 512)],
                                  start=(ct == 0),
                                  stop=(ct == C // P - 1),
                              )
                          nc.scalar.activation(
                              qT[ot][:, bass.ts(mh, 512)], ps[:], AF.Identity,
                              bias=bqsb[:, ot : ot + 1],
                          )

                  # ===== PE warm-up filler: keep the tensor engine busy through
                  # the AllGather wait so attention starts at full clock rate.
                  scratch_ps = eps.tile([P, 512], F32, name="warm_ps", tag="e_ps", bufs=4)
                  for w_ in range(24):
                      nc.tensor.matmul(
                          scratch_ps[:],
                          wqt[w_ % 2][:, 0:P],
                          h2T[w_ % 2][:, 0:512],
                          start=(w_ == 0),
                          stop=(w_ == 23),
                      )
                  warm_sb = apool.tile([1, 4], F32, name="warm_sb", bufs=1)
                  nc.vector.tensor_copy(out=warm_sb[:], in_=scratch_ps[0:1, 0:4])
                  warm_dram = dram.tile([4], F32, name="warm_dram")
                  nc.sync.dma_start(warm_dram[:].rearrange("(p f) -> p f", p=1), warm_sb[:])

          # ================= phase 5: attention over all 64 key tiles,
          # AV matmuls software-pipelined one tile behind eT/exp; pt bufs=8
          # lets eT/exp run ahead while the h2 AllGather finishes.
          nc.vector.memset(S[:], 0.0)
          if True:
              p2sb = apool.tile([P, C // P], F32, name="p2sb", bufs=1)
              for cc in range(C // P):
                  nc.vector.reduce_sum(
                      p2sb[:, cc : cc + 1], h2T[cc][:].bitcast(F32),
                      axis=mybir.AxisListType.X,
                  )
              nc.sync.dma_start(p2_ap.rearrange("(t p) -> p t", p=P), p2sb[:])
              av_ps = [avp.tile([P, M], F32, name=f"av{cc}", tag=f"av{cc}") for cc in range(C // P)]

              def emit_av(pt_, h2r_, t_, tg_):
                  for cc in range(C // P):
                      for mh in range(M // 512):
                          nc.tensor.matmul(
                              av_ps[cc][:, bass.ts(mh, 512)],
                              h2r_[:, t_ * C + cc * P : t_ * C + (cc + 1) * P],
                              pt_[:, bass.ts(mh, 512)],
                              start=(tg_ == 0),
                              stop=(tg_ == NT - 1),
                          )

              AV_DELAY = 10
              pending = []
              for r in range(NCORES):
                  if r not in kr_tiles:
                      load_kr(r)
                  kr = kr_tiles[r]
                  h2r = krpool.tile([P, NT_LOC * C], BF16, name="h2r")
                  h2r_dma = nc.sync.dma_start(
                      h2r[:].rearrange("p (t f) -> p t f", t=NT_LOC),
                      agoutH[r * HSHARD : (r + 1) * HSHARD].rearrange(
                          "(t p f) -> p t f", t=NT_LOC, p=P
                      ),
                  )
                  if r == 0:
                      # keep the first eT's k loads ahead of the 1MB h2 load
                      for d_ in kr_dmas[:4]:
                          add_dep_helper(h2r_dma.ins, d_.ins, sync=False,
                                         reason="rank0 h2r after entry kr loads")
                  for t in range(NT_LOC):
                      tg = r * NT_LOC + t
                      pt = apool.tile([P, M], BF16, name="pt", bufs=16)
                      for mh in range(M // 512):
                          e_ps = eps.tile([P, 512], F32, name="e_ps", tag="e_ps", bufs=4)
                          for ot in range(C // P):
                              nc.tensor.matmul(
                                  e_ps[:],
                                  kr[ot][:, bass.ts(t, P)],
                                  qT[ot][:, bass.ts(mh, 512)],
                                  start=(ot == 0),
                                  stop=(ot == C // P - 1),
                              )
                          nc.scalar.activation(pt[:, bass.ts(mh, 512)], e_ps[:], AF.Exp)
                      nc.vector.tensor_tensor(S[:], S[:], pt[:], ALU.add)
                      pending.append((pt, h2r, t, tg))
                      if len(pending) > AV_DELAY:
                          emit_av(*pending.pop(0))
              for args in pending:
                  emit_av(*args)

              # ===== epilogue: rowsum -> reciprocal -> scale -> partial sums
              rs_full = eps.tile([P, 512], F32, name="rs_ps", tag="e_ps", bufs=4)
              rs_full2 = eps.tile([P, 512], F32, name="rs_ps2", tag="e_ps", bufs=4)
              rs_halves = [rs_full[0:1, :], rs_full2[0:1, :]]
              for mh in range(M // 512):
                  nc.tensor.matmul(
                      rs_halves[mh][:],
                      ones_f[:],
                      S[:, bass.ts(mh, 512)],
                      start=True,
                      stop=True,
                  )
              recip = apool.tile([1, M], F32, name="recip", bufs=1)
              recipb = apool.tile([P, M], F32, name="recipb", bufs=1)
              for mh in range(M // 512):
                  nc.vector.reciprocal(recip[:, bass.ts(mh, 512)], rs_halves[mh][:])
                  nc.gpsimd.partition_broadcast(
                      recipb[:, bass.ts(mh, 512)], recip[:, bass.ts(mh, 512)]
                  )

              p1sb = apool.tile([P, C // P], F32, name="p1sb", bufs=1)
              p1h = apool.tile([P, 2 * (C // P)], F32, name="p1h", bufs=1)
              for cc in range(C // P):
                  avn = apool.tile([P, M], F32, name="avn", bufs=2)
                  for mh in range(M // 512):
                      j = cc * 2 + mh
                      nc.vector.tensor_tensor(
                          avn[:, bass.ts(mh, 512)],
                          av_ps[cc][:, bass.ts(mh, 512)],
                          recipb[:, bass.ts(mh, 512)], ALU.mult,
                      )
                      nc.vector.reduce_sum(
                          p1h[:, j : j + 1], avn[:, bass.ts(mh, 512)],
                          axis=mybir.AxisListType.X,
                      )
                  nc.vector.tensor_tensor(
                      p1sb[:, cc : cc + 1], p1h[:, cc * 2 : cc * 2 + 1],
                      p1h[:, cc * 2 + 1 : cc * 2 + 2], ALU.add,
                  )
              nc.sync.dma_start(p1_ap.rearrange("(t p) -> p t", p=P), p1sb[:])


def build_nc(collective=True, repeat=1):
    nc = bacc.Bacc("TRN2", target_bir_lowering=False, debug=False, num_devices=NCORES)
    ins = {}

    def di(name, shape, dt):
        ins[name] = nc.dram_tensor(name, list(shape), dt, kind="ExternalInput").ap()

    di("xT", (D, M), BF16)
    di("w1t", (D, H), BF16)
    di("w2t", (H, C), F32R)
    di("wqt", (C, C), F32R)
    di("wkt", (C, C), F32R)
    di("b2r", (1, C), F32R)
    di("ones_r", (1, P), F32R)
    di("ones_f", (P, 1), F32)
    di("b1f", (H,), F32)
    di("b2f", (C,), F32)
    di("bqf", (C,), F32)
    di("bkf", (C,), F32)
    p1_ap = nc.dram_tensor("partial_av", [C], F32, kind="ExternalOutput").ap()
    p2_ap = nc.dram_tensor("partial_res", [C], F32, kind="ExternalOutput").ap()

    with tile.TileContext(nc) as tc:
        for _ in range(repeat):
            _body(nc, tc, ins, p1_ap, p2_ap, collective)
    nc.compile()
    return nc


_CACHE = {}


def _get_nc(collective=True, repeat=1):
    key = (collective, repeat)
    if key not in _CACHE:
        _CACHE[key] = build_nc(collective=collective, repeat=repeat)
    return _CACHE[key]


def make_in_maps(x, W1, b1, W2, b2, Wq, bq, Wk, bk):
    import ml_dtypes
    xT = np.ascontiguousarray(np.asarray(x, np.float32)[0].T.astype(ml_dtypes.bfloat16))
    w1t = np.ascontiguousarray(np.asarray(W1, np.float32).T.astype(ml_dtypes.bfloat16))
    w2t = np.ascontiguousarray(np.asarray(W2, np.float32).T)        # (H, C)
    wqt = np.ascontiguousarray(np.asarray(Wq, np.float32).T)        # (C, C)
    wkt = np.ascontiguousarray(np.asarray(Wk, np.float32).T)        # (C, C)
    common = {
        "w1t": w1t, "w2t": w2t, "wqt": wqt, "wkt": wkt,
        "b2r": np.asarray(b2, np.float32).reshape(1, C),
        "ones_r": np.ones((1, P), np.float32),
        "ones_f": np.ones((P, 1), np.float32),
        "b1f": np.asarray(b1, np.float32),
        "b2f": np.asarray(b2, np.float32),
        "bqf": np.asarray(bq, np.float32),
        "bkf": np.asarray(bk, np.float32),
    }
    return [
        {"xT": np.ascontiguousarray(xT[:, r * M : (r + 1) * M]), **common}
        for r in range(NCORES)
    ]


def finish(results, gamma, W3, b3):
    p1 = np.sum([r["partial_av"] for r in results], axis=0, dtype=np.float64)
    p2 = np.sum([r["partial_res"] for r in results], axis=0, dtype=np.float64)
    g = float(np.asarray(gamma).reshape(-1)[0])
    x2 = ((g * p1 + p2) / N).astype(np.float32)
    logits = x2 @ np.asarray(W3, np.float32).T + np.asarray(b3, np.float32)
    return logits[None, :].astype(np.float32)


def kernel(x, W1, b1, W2, b2, Wq, bq, Wk, bk, gamma, W3, b3):
    nc = _get_nc(collective=True, repeat=1)
    in_maps = make_in_maps(x, W1, b1, W2, b2, Wq, bq, Wk, bk)
    res = bass_utils.run_bass_kernel_spmd(
        nc, in_maps, core_ids=list(range(NCORES)), trace=False
    )
    return finish(res.results, gamma, W3, b3)

